# revision 3
# baseline (speedup 1.0000x reference)
"""Trainium2 Bass kernel for nn_EncoderRNN (3-layer 'bidirectional' GRU,
B=128, T=64, IN=256, H=1024; both directions run forward in time).

Sharding (8 cores): dir x gate-slice tensor parallel.
  core c: dir = c//4, q = c%4 -> owns h-columns [q*256, (q+1)*256) of its
  direction, i.e. 768 gate columns (r|z|n) of that dir.

Per (layer l, time t) step, each core computes its gate slice for the full
batch as a PSUM accumulation (batch on partitions, gates on free dim):
  psum[:, 0:512)   = bias_rz + x_t @ Wih_rz + h_{t-1} @ Whh_rz
  psum[:, 512:768) = bias_in + x_t @ Wih_n
  psum[:, 768:1024)= bias_hn + h_{t-1} @ Whh_n
then r,z = sigmoid, n = tanh(i_n + r*h_n), h_new = n + z*(h-n).
h_new is transposed on the PE and AllGathered (partition-concat) so every
core gets the full h^T it needs: 4-rank (within dir) fp32r for the next
recurrent step, 8-rank bf16 for the next layer's input projection.

Matmuls run in fp32r (full PE rate at moving-dim >= 256, ~1e-4 matmul
rel err) except the layer-1/2 input projections which use bf16 weights
(SBUF capacity). Wavefront schedule: wall step s runs layer l at t = s-l,
so AllGather latency hides under the other layers' PE work.

The kernel returns each core's (B, 256) slice of hidden[0] (the
seq_len-gathered top-layer output); the host assembles the (1, B, 2H)
result.
"""

import numpy as np
import ml_dtypes

from concourse import bacc, bass, tile, mybir
from concourse import bass_utils

F32 = mybir.dt.float32
F32R = mybir.dt.float32r
BF16 = mybir.dt.bfloat16

B, T, IN, H, L = 128, 64, 256, 1024, 3
NC = 8
SL = 256          # h-columns per core (per its dir)
GW = 3 * SL       # gate columns per core: [r | z | n]
AF = mybir.ActivationFunctionType
ALU = mybir.AluOpType


TAP_STEPS = {(0, 0), (0, 1), (1, 0), (2, 0), (0, 2), (1, 1)}


def _emit(nc, tc, pools, state):
    """Emit the full wavefront program."""
    const, work, hnp = pools["const"], pools["work"], pools["hnp"]
    pgp, ptp, dram = pools["pg"], pools["pt"], pools["dram"]
    inp = state["inputs"]

    # --- resident tiles -------------------------------------------------
    wih_sb = []
    whh_sb = []
    bias_sb = []
    for l in range(L):
        kt = 2 if l == 0 else 16
        wdt = F32R if l == 0 else BF16
        w = const.tile([128, kt, 768], wdt, tag=f"wih{l}")
        nc.sync.dma_start(
            w[:], inp[f"wih{l}"].ap().rearrange("(k p) n -> p k n", p=128)
        )
        wih_sb.append(w)
        wh = const.tile([128, 8, 768], F32R, tag=f"whh{l}")
        nc.sync.dma_start(
            wh[:], inp[f"whh{l}"].ap().rearrange("(k p) n -> p k n", p=128)
        )
        whh_sb.append(wh)
    bias_all = const.tile([1, 3 * 1024], F32R, tag="bias")
    nc.sync.dma_start(bias_all[:], inp["bias"].ap())
    for l in range(L):
        bias_sb.append(bias_all[:, l * 1024 : (l + 1) * 1024])
    ones = const.tile([1, 128], F32R, tag="ones")
    nc.sync.dma_start(ones[:], inp["ones"].ap())
    ident = const.tile([128, 128], F32, tag="ident")
    nc.sync.dma_start(ident[:], inp["ident"].ap())
    mask_sb = const.tile([128, T], F32, tag="mask")
    nc.sync.dma_start(mask_sb[:], inp["mask"].ap())

    zeros = const.tile([128, SL], F32, tag="zeros")
    nc.vector.memset(zeros[:], 0.0)
    hsel = const.tile([128, SL], F32, tag="hsel")
    nc.vector.memset(hsel[:], 0.0)

    # per-layer rolling state
    h_prev = [None] * L          # SBUF (128, SL) fp32, h_{t-1} (own slice)
    agr_out = [None] * L         # DRAM (1024,128) fp32r: own-dir h^T at t-1
    agb_out = [None] * L         # DRAM (2048,128) bf16: full h^T at t (l<2)

    def emit_step(l, t):
        # ---- lhsT loads ----
        if l == 0:
            xt = work.tile([128, 2, 128], F32R, tag="xt")
            nc.sync.dma_start(
                xt[:], inp["xT"].ap()[t].rearrange("(k p) n -> p k n", p=128)
            )
            gi_lhs, n_gik = xt, 2
        else:
            xb = work.tile([128, 16, 128], BF16, tag=f"xb{l}")
            nc.sync.dma_start(
                xb[:], agb_out[l - 1].rearrange("(k p) n -> p k n", p=128)
            )
            gi_lhs, n_gik = xb, 16
        if t > 0:
            hr = work.tile([128, 8, 128], F32R, tag=f"hr{l}")
            nc.sync.dma_start(
                hr[:], agr_out[l].rearrange("(k p) n -> p k n", p=128)
            )

        # ---- matmuls into psum ----
        pg = pgp.tile([128, 1024], F32, tag="pg")
        wih, whh, bs = wih_sb[l], whh_sb[l], bias_sb[l]
        # region rz [0:512)
        nc.tensor.matmul(pg[:, 0:512], ones[:], bs[:, 0:512], start=True, stop=False)
        for k in range(n_gik):
            nc.tensor.matmul(
                pg[:, 0:512], gi_lhs[:, k, :], wih[:, k, 0:512],
                start=False, stop=(t == 0 and k == n_gik - 1),
            )
        if t > 0:
            for k in range(8):
                nc.tensor.matmul(
                    pg[:, 0:512], hr[:, k, :], whh[:, k, 0:512],
                    start=False, stop=(k == 7),
                )
        # region gi_n [512:768)
        nc.tensor.matmul(
            pg[:, 512:768], ones[:], bs[:, 512:768], start=True, stop=False
        )
        for k in range(n_gik):
            nc.tensor.matmul(
                pg[:, 512:768], gi_lhs[:, k, :], wih[:, k, 512:768],
                start=False, stop=(k == n_gik - 1),
            )
        # region gh_n [768:1024)
        nc.tensor.matmul(
            pg[:, 768:1024], ones[:], bs[:, 768:1024], start=True, stop=(t == 0)
        )
        if t > 0:
            for k in range(8):
                nc.tensor.matmul(
                    pg[:, 768:1024], hr[:, k, :], whh[:, k, 512:768],
                    start=False, stop=(k == 7),
                )

        # ---- elementwise ----
        hp = h_prev[l] if t > 0 else zeros
        rz = work.tile([128, 512], F32, tag="rz")
        nc.scalar.activation(rz[:], pg[:, 0:512], AF.Sigmoid)
        rhn = work.tile([128, SL], F32, tag="rhn")
        nc.vector.tensor_tensor(rhn[:], rz[:, 0:SL], pg[:, 768:1024], ALU.mult)
        npre = work.tile([128, SL], F32, tag="npre")
        nc.vector.tensor_tensor(npre[:], rhn[:], pg[:, 512:768], ALU.add)
        n_t = work.tile([128, SL], F32, tag="n_t")
        nc.scalar.activation(n_t[:], npre[:], AF.Tanh)
        delta = work.tile([128, SL], F32, tag="delta")
        nc.vector.tensor_tensor(delta[:], hp[:], n_t[:], ALU.subtract)
        zd = work.tile([128, SL], F32, tag="zd")
        nc.vector.tensor_tensor(zd[:], rz[:, SL:512], delta[:], ALU.mult)
        hn = hnp.tile([128, SL], F32, tag="hn")
        nc.vector.tensor_tensor(hn[:], n_t[:], zd[:], ALU.add)
        h_prev[l] = hn
        taps = state.get("taps")
        if taps is not None and (l, t) in TAP_STEPS:
            nc.sync.dma_start(taps[f"tap_h_{l}_{t}"].ap(), hn[:])
            nc.sync.dma_start(taps[f"tap_rz_{l}_{t}"].ap(), rz[:])
            if t > 0:
                nc.sync.dma_start(
                    taps[f"tap_hr_{l}_{t}"].ap(),
                    hr[:].rearrange("p k n -> p (k n)"),
                )

        if l == L - 1:
            d1 = work.tile([128, SL], F32, tag="rhn")
            nc.vector.tensor_tensor(d1[:], hn[:], hsel[:], ALU.subtract)
            d1m = work.tile([128, SL], F32, tag="npre")
            nc.vector.tensor_scalar_mul(d1m[:], d1[:], mask_sb[:, t : t + 1])
            nc.vector.tensor_tensor(hsel[:], hsel[:], d1m[:], ALU.add)

        # ---- transpose + allgathers ----
        last = t == T - 1
        need_r = not last                 # next own-layer recurrent step
        need_b = (l < L - 1)              # next layer's gi at this t
        if not (need_r or need_b):
            return
        pt = ptp.tile([128, 256], F32, tag="pt")
        nc.tensor.transpose(pt[:, 0:128], hn[:, 0:128], ident[:])
        nc.tensor.transpose(pt[:, 128:256], hn[:, 128:256], ident[:])
        if need_r:
            htr = work.tile([128, 2, 128], F32R, tag="htr")
            nc.vector.tensor_copy(htr[:], pt[:].rearrange("p (k n) -> p k n", k=2))
            agr_in = dram.tile([256, 128], F32R, tag="agr_in")
            nc.sync.dma_start(
                agr_in.rearrange("(k p) n -> p k n", p=128), htr[:]
            )
            out_r = dram.tile([1024, 128], F32R, tag="agr_out")
            nc.gpsimd.collective_compute(
                "AllGather",
                ALU.bypass,
                replica_groups=[[0, 1, 2, 3], [4, 5, 6, 7]],
                ins=[agr_in.opt()],
                outs=[out_r.opt()],
            )
            agr_out[l] = out_r
        if need_b:
            htb = work.tile([128, 2, 128], BF16, tag="htb")
            nc.vector.tensor_copy(htb[:], pt[:].rearrange("p (k n) -> p k n", k=2))
            agb_in = dram.tile([256, 128], BF16, tag="agb_in")
            nc.sync.dma_start(
                agb_in.rearrange("(k p) n -> p k n", p=128), htb[:]
            )
            out_b = dram.tile([2048, 128], BF16, tag="agb_out")
            nc.gpsimd.collective_compute(
                "AllGather",
                ALU.bypass,
                replica_groups=[[0, 1, 2, 3, 4, 5, 6, 7]],
                ins=[agb_in.opt()],
                outs=[out_b.opt()],
            )
            agb_out[l] = out_b

    for s in range(T + L - 1):
        for l in reversed(range(L)):
            t = s - l
            if 0 <= t < T:
                emit_step(l, t)

    nc.sync.dma_start(state["out_ap"], hsel[:])


def build_nc(taps=False):
    nc = bacc.Bacc("TRN2", target_bir_lowering=False, debug=False, num_devices=NC)
    inputs = {}
    inputs["xT"] = nc.dram_tensor("xT", [T, IN, B], F32R, kind="ExternalInput")
    inputs["wih0"] = nc.dram_tensor("wih0", [IN, 768], F32R, kind="ExternalInput")
    inputs["wih1"] = nc.dram_tensor("wih1", [2048, 768], BF16, kind="ExternalInput")
    inputs["wih2"] = nc.dram_tensor("wih2", [2048, 768], BF16, kind="ExternalInput")
    for l in range(L):
        inputs[f"whh{l}"] = nc.dram_tensor(
            f"whh{l}", [H, 768], F32R, kind="ExternalInput"
        )
    inputs["bias"] = nc.dram_tensor("bias", [1, L * 1024], F32R, kind="ExternalInput")
    inputs["ones"] = nc.dram_tensor("ones", [1, 128], F32R, kind="ExternalInput")
    inputs["ident"] = nc.dram_tensor("ident", [128, 128], F32, kind="ExternalInput")
    inputs["mask"] = nc.dram_tensor("mask", [B, T], F32, kind="ExternalInput")
    out = nc.dram_tensor("hsel", [B, SL], F32, kind="ExternalOutput")
    tap_tensors = None
    if taps:
        tap_tensors = {}
        for (l, t) in TAP_STEPS:
            tap_tensors[f"tap_h_{l}_{t}"] = nc.dram_tensor(
                f"tap_h_{l}_{t}", [B, SL], F32, kind="ExternalOutput"
            )
            tap_tensors[f"tap_rz_{l}_{t}"] = nc.dram_tensor(
                f"tap_rz_{l}_{t}", [B, 512], F32, kind="ExternalOutput"
            )
            if t > 0:
                tap_tensors[f"tap_hr_{l}_{t}"] = nc.dram_tensor(
                    f"tap_hr_{l}_{t}", [B, 1024], F32R, kind="ExternalOutput"
                )

    with tile.TileContext(nc) as tc:
        with (
            tc.tile_pool(name="const", bufs=1) as const,
            tc.tile_pool(name="work", bufs=2) as work,
            tc.tile_pool(name="hnp", bufs=6) as hnp,
            tc.tile_pool(name="pg", bufs=3, space="PSUM") as pgp,
            tc.tile_pool(name="pt", bufs=2, space="PSUM") as ptp,
            tc.tile_pool(name="dram", bufs=6, space="DRAM") as dram,
        ):
            pools = {"const": const, "work": work, "hnp": hnp,
                     "pg": pgp, "pt": ptp, "dram": dram}
            state = {"inputs": inputs, "out_ap": out.ap(), "taps": tap_tensors}
            _emit(nc, tc, pools, state)
    nc.compile()
    return nc


def make_in_maps(input_tensor, seq_len, w_ih, w_hh, b_ih, b_hh):
    """Host-side prep: per-core weight slices / layouts."""
    x = np.asarray(input_tensor, dtype=np.float32)
    sl = np.asarray(seq_len, dtype=np.int32)
    w_ih = [np.asarray(w, dtype=np.float32) for w in w_ih]
    w_hh = [np.asarray(w, dtype=np.float32) for w in w_hh]
    b_ih = [np.asarray(b, dtype=np.float32) for b in b_ih]
    b_hh = [np.asarray(b, dtype=np.float32) for b in b_hh]

    xT = np.ascontiguousarray(x.transpose(1, 2, 0))  # (T, IN, B)
    mask = (sl[:, None] - 1 == np.arange(T)[None, :]).astype(np.float32)  # (B,T)
    ident = np.eye(128, dtype=np.float32)
    ones = np.ones((1, 128), dtype=np.float32)

    in_maps = []
    for c in range(NC):
        d, q = divmod(c, 4)
        j0 = q * SL
        idx = np.r_[j0 : j0 + SL, H + j0 : H + j0 + SL, 2 * H + j0 : 2 * H + j0 + SL]
        m = {
            "xT": xT,
            "ident": ident,
            "ones": ones,
            "mask": mask,
        }
        bias = np.zeros((1, L * 1024), dtype=np.float32)
        for l in range(L):
            wih_c = np.ascontiguousarray(w_ih[l][d][idx, :].T)  # (d_in, 768)
            whh_c = np.ascontiguousarray(w_hh[l][d][idx, :].T)  # (H, 768)
            m[f"wih{l}"] = (
                wih_c if l == 0 else wih_c.astype(ml_dtypes.bfloat16)
            )
            m[f"whh{l}"] = whh_c
            bsum = b_ih[l][d] + b_hh[l][d]
            o = l * 1024
            bias[0, o : o + 512] = bsum[idx[: 2 * SL]]            # rz combined
            bias[0, o + 512 : o + 768] = b_ih[l][d][idx[2 * SL :]]   # i_n bias
            bias[0, o + 768 : o + 1024] = b_hh[l][d][idx[2 * SL :]]  # h_n bias
        m["bias"] = bias
        in_maps.append(m)
    return in_maps


def assemble(results):
    out = np.zeros((1, B, 2 * H), dtype=np.float32)
    for c in range(NC):
        d, q = divmod(c, 4)
        j0 = d * H + q * SL
        out[0, :, j0 : j0 + SL] = results[c]["hsel"]
    return out


_NC_CACHE = None


def kernel(input_tensor, seq_len, w_ih, w_hh, b_ih, b_hh):
    global _NC_CACHE
    if _NC_CACHE is None:
        _NC_CACHE = build_nc()
    in_maps = make_in_maps(input_tensor, seq_len, w_ih, w_hh, b_ih, b_hh)
    res = bass_utils.run_bass_kernel_spmd(
        _NC_CACHE, in_maps, core_ids=list(range(NC))
    )
    return assemble(res.results)
